# revision 15
# baseline (speedup 1.0000x reference)
"""BitLinear (2-bit packed weights) matmul kernel for 8 TRN2 NeuronCores.

Computation (per reference):
  s   = 127 / clip(rowmax|x|, 1e-5)            # [M,1]
  q   = round(x * s)                           # int-valued, |q| <= 127
  w   = unpack2bit(weight) - 1                 # [N,K], values {-1,0,1,2}
  acc = q @ w.T                                # exact in bf16 matmul + f32 PSUM
  out = acc / s * ws[n % 4]   -> bf16

Sharding: tensor-parallel along N (out_features). Each of 8 cores gets
weight rows [c*1376, (c+1)*1376), full x, full weight_scale; computes its
[M, 1376] output column block; host concatenates along axis 1.

Dataflow per core:
  - weight prep (one-time): DMA packed int32, unpack codes with int16
    shift/and on DVE, subtract-1-and-cast to bf16 in place, then xbar DMA
    transpose into wT tiles laid out [128k, n_tile, 32kt, 128n].
  - per 128-row x block: DMA x, abs-max reduce, scale via ScalarE with the
    +1.5*2^23 round-to-nearest trick, cast to bf16, xbar DMA transpose to
    [128k, 32kt, 128m], then 32x4 accumulating matmuls into PSUM and a
    fused (acc * 1/s * ws) epilogue on DVE.
"""

import numpy as np

import concourse.bass as bass
from concourse import bacc, mybir
from concourse.tile import TileContext

M, K, N = 8192, 4096, 11008
N_CORES = 8
N_SHARD = N // N_CORES  # 1376
MAGIC = 12582912.0  # 1.5 * 2**23 : float32 RNE rounding trick


def build_kernel(m=M, k=K, n_shard=N_SHARD):
    kp = k // 4           # packed columns
    nkt = k // 128        # k-tiles (contraction)
    nmb = m // 128        # m row blocks
    nnt = (n_shard + 127) // 128  # n tiles for weight prep

    nc = bacc.Bacc()
    x_ext = nc.declare_dram_parameter("x", [m, k], mybir.dt.float32, isOutput=False)
    w_ext = nc.declare_dram_parameter(
        "weight", [n_shard, kp], mybir.dt.int32, isOutput=False
    )
    ws_ext = nc.declare_dram_parameter(
        "weight_scale", [4], mybir.dt.float32, isOutput=False
    )
    out_ext = nc.declare_dram_parameter(
        "out", [m, n_shard], mybir.dt.bfloat16, isOutput=True
    )

    # output chunks of up to 4 n-tiles (<=512 f32, one PSUM bank each);
    # the last chunk is padded to a whole number of 128-wide n-tiles
    chunk_nts = []
    t0 = 0
    while t0 < nnt:
        chunk_nts.append((t0, min(4, nnt - t0)))
        t0 += 4
    n_pad = nnt * 128  # psum width incl. padding (1408)

    with TileContext(nc) as tc:
        with (
            tc.tile_pool(name="const", bufs=1) as cpool,
            tc.tile_pool(name="wt", bufs=1) as wtpool,
            tc.tile_pool(name="wprep", bufs=3) as wppool,
            tc.tile_pool(name="xp", bufs=2) as xpool,
            tc.tile_pool(name="qn", bufs=2) as qnpool,
            tc.tile_pool(name="qt", bufs=3) as qtpool,
            tc.tile_pool(name="osb", bufs=2) as opool,
            tc.tile_pool(name="sc", bufs=3) as spool,
            tc.tile_pool(name="psacc", bufs=2, space="PSUM") as psacc,
        ):
            ws128 = cpool.tile([128, 4], mybir.dt.float32)
            nc.sync.dma_start(
                out=ws128[:, :],
                in_=ws_ext[:].unsqueeze(0).broadcast_to([128, 4]),
            )

            # ---- weight prep: unpack 2-bit codes, transpose to [k, n] ----
            # one wT tile per psum chunk so early chunks unblock matmuls
            # before the full weight prep finishes
            wTs = [
                wtpool.tile(
                    [128, tcnt, nkt, 128],
                    mybir.dt.bfloat16,
                    tag=f"wt{ci}",
                    name=f"wT{ci}",
                )
                for ci, (_, tcnt) in enumerate(chunk_nts)
            ]
            for t in range(nnt):
                rows = min(128, n_shard - t * 128)
                wp = wppool.tile([128, kp], mybir.dt.int32, tag="wprep")
                nc.sync.dma_start(
                    out=wp[:rows, :], in_=w_ext[t * 128 : t * 128 + rows, :]
                )
                if rows < 128:
                    # pad rows produce harmless values in psum cols >= n_shard
                    nc.vector.memset(wp[rows:, :], 0)
                # int16 view of the packed words: low halfword holds the byte
                wp16 = wp.bitcast(mybir.dt.int16).rearrange(
                    "p (c two) -> p c two", two=2
                )
                wi = wppool.tile([128, k], mybir.dt.int16, tag="wprep")
                wi4 = wi.rearrange("p (c four) -> p c four", four=4)
                for i in range(4):
                    # codes 0..3 = (packed >> 2i) & 3  (bitwise ops can't
                    # cast, so stage as int16 = xbar-transposable width)
                    nc.vector.tensor_scalar(
                        out=wi4[:, :, i : i + 1],
                        in0=wp16[:, :, 0:1],
                        scalar1=2 * i,
                        scalar2=3,
                        op0=mybir.AluOpType.logical_shift_right,
                        op1=mybir.AluOpType.bitwise_and,
                    )
                # codes-1 in {-1,0,1,2}, cast to bf16 in place
                wn = wi.bitcast(mybir.dt.bfloat16)
                nc.vector.tensor_scalar_sub(wn[:, :], wi[:, :], 1)
                ci, tloc = t // 4, t % 4
                nc.sync.dma_start_transpose(wTs[ci][:, tloc, :, :], wn[:, :])

            # ---- main loop over 128-row blocks of x ----
            for b in range(nmb):
                xt = xpool.tile([128, k], mybir.dt.float32, tag="xp")
                nc.sync.dma_start(out=xt[:, :], in_=x_ext[b * 128 : (b + 1) * 128, :])

                r = spool.tile([128, 1], mybir.dt.float32, tag="r")
                nc.vector.tensor_reduce(
                    out=r[:, :],
                    in_=xt[:, :],
                    axis=mybir.AxisListType.X,
                    op=mybir.AluOpType.max,
                    apply_absolute_value=True,
                )
                rc = spool.tile([128, 1], mybir.dt.float32, tag="rc")
                nc.vector.tensor_scalar_max(rc[:, :], r[:, :], 1e-5)
                rinv = spool.tile([128, 1], mybir.dt.float32, tag="rinv")
                nc.vector.reciprocal(rinv[:, :], rc[:, :])
                s_t = spool.tile([128, 1], mybir.dt.float32, tag="s")
                nc.vector.tensor_scalar_mul(s_t[:, :], rinv[:, :], 127.0)
                rs_t = spool.tile([128, 1], mybir.dt.float32, tag="rs")
                nc.vector.tensor_scalar_mul(rs_t[:, :], rc[:, :], 1.0 / 127.0)

                # x <- x*s + MAGIC (f32 add rounds to integer), then q = x - MAGIC
                nc.scalar.activation(
                    xt[:, :],
                    xt[:, :],
                    mybir.ActivationFunctionType.Copy,
                    bias=MAGIC,
                    scale=s_t[:, 0:1],
                )
                qn = qnpool.tile([128, k], mybir.dt.bfloat16, tag="qn")
                nc.vector.tensor_scalar_sub(qn[:, :], xt[:, :], MAGIC)

                qT = qtpool.tile([128, nkt, 128], mybir.dt.bfloat16, tag="qt")
                nc.sync.dma_start_transpose(qT[:, :, :], qn[:, :])

                pacc = psacc.tile([128, n_pad], mybir.dt.float32)
                for kt in range(nkt):
                    for ci, (ct0, tcnt) in enumerate(chunk_nts):
                        nc.tensor.matmul(
                            pacc[:, ct0 * 128 : (ct0 + tcnt) * 128],
                            lhsT=qT[:, kt, :],
                            rhs=wTs[ci][:, :, kt, :],
                            start=(kt == 0),
                            stop=(kt == nkt - 1),
                        )

                osb = opool.tile([128, n_shard], mybir.dt.bfloat16)
                nc.vector.scalar_tensor_tensor(
                    out=osb.rearrange("p (c four) -> p c four", four=4),
                    in0=pacc[:, :n_shard].rearrange("p (c four) -> p c four", four=4),
                    scalar=rs_t[:, 0:1],
                    in1=ws128[:, :].unsqueeze(1).broadcast_to([128, n_shard // 4, 4]),
                    op0=mybir.AluOpType.mult,
                    op1=mybir.AluOpType.mult,
                )
                nc.sync.dma_start(
                    out=out_ext[b * 128 : (b + 1) * 128, :], in_=osb[:, :]
                )

    return nc


def kernel(x, weight, weight_scale):
    from concourse.bass_utils import run_bass_kernel_spmd

    nc = build_kernel()
    nc.finalize()
    in_maps = [
        {
            "x": np.ascontiguousarray(x, dtype=np.float32),
            "weight": np.ascontiguousarray(
                weight[c * N_SHARD : (c + 1) * N_SHARD, :], dtype=np.int32
            ),
            "weight_scale": np.ascontiguousarray(weight_scale, dtype=np.float32),
        }
        for c in range(N_CORES)
    ]
    res = run_bass_kernel_spmd(nc, in_maps, core_ids=list(range(N_CORES)))
    out = np.concatenate([res.results[c]["out"] for c in range(N_CORES)], axis=1)
    return out
